# revision 8
# baseline (speedup 1.0000x reference)
"""Trainium2 Bass kernel for per-edge dot products (GNN DotPredictor).

out[e] = sum(h[src[e]] * h[dst[e]]); 800k edges, h [50k, 64] f32, 8 cores.

Design (v4):
  - Edges sharded 8 ways; h replicated. Per-edge rows fetched from HBM with
    the Q7 `dma_gather` path. The Q7 descriptor generation (~8ns/descriptor
    per cpu pair) is the bottleneck, so it is parallelized 4x across the 4
    SWDGE queues (each queue's descriptors are generated by its own Q7 cpu
    pair) and minimized:
      * edges sorted by src; equal-src runs decomposed into K-edge units
        (K in {16,8,4,2,1}); one 256B src descriptor serves K edges (hu
        broadcast via step-0 AP).
      * dst side: one 512B descriptor per edge covering the node-row PAIR
        (2*(d//2), 2*(d//2)+1), so the int16 index (d//2 < 25004) spans the
        whole table and src runs aren't fragmented by a dst-range bucket.
        The DVE computes dots against BOTH rows ([.., 2K] candidates); the
        host selects the d%2 candidate during un-permutation (free).
  - src-side int16 index is (src - s_hi*32768) with per-range base pointers;
    units are grouped into (K, s_hi) buckets so each chunk has one base.
  - DVE: hu broadcast across 2K candidate slots via step-0 AP, in-place
    multiply into the hv tile, segment-reduce the 64-feature dim to the two
    candidate scores per edge.
  - Output [128, 2*tiles] stored contiguously; host transposes + selects.
"""

import os
from contextlib import ExitStack

import numpy as np

import concourse.bacc as bacc
import concourse.mybir as mybir
from concourse import library_config
from concourse.bass import AP
from concourse._compat import get_trn_type
from concourse.bass_utils import run_bass_kernel_spmd

N_NODES = 50000
NPAD = 50008  # even; pair reads at d//2=25003 end at row 50007
D = 64
P = 128
N_CORES = 8
SPLIT = 32768
NQ = 4  # SWDGE queues (each with its own Q7 descriptor-gen cpu pair)
NB = 6  # buffer pairs

KS = (16, 8, 4, 2, 1)
G_MAP = {16: 256, 8: 512, 4: 1024, 2: 2048, 1: 2048}  # units per chunk
QFIX = 112  # per-instruction fixed cost in descriptor-equivalents

TRACE = False
LAST_RESULT = None


def _ensure_ntff_hook():
    """bass_utils' trace path imports antenv.axon_hooks, which this image's
    antenv package lacks. Recreate it from the boot helper so trace=True
    works; harmless no-op if the real module exists."""
    import sys
    import types

    try:
        import antenv.axon_hooks  # noqa: F401

        return
    except ImportError:
        pass
    try:
        import antenv
        from trn_agent_boot.trn_boot import _ntff_profile_via_ctypes

        hook = _ntff_profile_via_ctypes("/opt/axon/libaxon_pjrt.so")
        m = types.ModuleType("antenv.axon_hooks")
        m.get_axon_ntff_profile_hook = lambda: hook
        m.set_axon_ntff_profile_hook = lambda h: None
        sys.modules["antenv.axon_hooks"] = m
        antenv.axon_hooks = m
    except Exception:
        pass


def _wrap_idx(vals):
    """int16 index array [Npc] -> the [128, Npc/16] SBUF layout dma_gather
    expects (idx i at partition i%16, column i//16, replicated over the 8
    groups of 16 partitions — each SWDGE queue's Q7 pair reads its own
    group)."""
    w = vals.reshape(-1, 16).T  # [16, Npc/16]
    return np.ascontiguousarray(np.tile(w, (8, 1)))  # [128, Npc/16]


def _host_prep(src, dst):
    """Sort by src; decompose equal-src runs into K-units; bucket units by
    (K, s_hi) so each chunk's src gather has one base pointer.

    Returns (schedule, seqs, pars, sidx, didx, u_total, e_total):
      schedule: list of (K, s_hi, u_off, e_off, n_units), same all cores
      seqs: [N_CORES, e_total] global edge id per edge slot (-1 pad)
      pars: [N_CORES, e_total] dst parity (d%2) per edge slot
    """
    E = src.shape[0]
    order0 = np.argsort(src, kind="stable")
    ss, sd = src[order0], dst[order0]

    new = np.ones(E, bool)
    new[1:] = ss[1:] != ss[:-1]
    run_start = np.flatnonzero(new)
    d = np.diff(np.append(run_start, E))
    run_id = np.cumsum(new) - 1
    r = np.arange(E) - run_start[run_id]
    dd = d[run_id]

    K_e = np.zeros(E, np.int64)
    m_e = np.zeros(E, np.int64)
    prev = np.zeros(E, np.int64)
    for K in KS:
        nK = prev + ((dd - prev) // K) * K
        seg = (r >= prev) & (r < nK)
        K_e[seg] = K
        m_e[seg] = (r - prev)[seg] % K
        prev = nK
    first = m_e == 0
    s_hi_e = ss >= SPLIT

    pad_units = N_CORES * P
    schedule = []
    sidx_parts = [[] for _ in range(N_CORES)]
    didx_parts = [[] for _ in range(N_CORES)]
    seq_parts = [[] for _ in range(N_CORES)]
    par_parts = [[] for _ in range(N_CORES)]
    u_off = 0
    e_off = 0
    for K in KS:
        for s_hi in (False, True):
            starts = np.flatnonzero(first & (K_e == K) & (s_hi_e == s_hi))
            if starts.size == 0:
                continue
            Upad = -(-starts.size // pad_units) * pad_units
            buf = np.full(Upad, -1, dtype=np.int64)
            buf[: starts.size] = starts
            U = Upad // N_CORES  # per-core units, multiple of 128
            for c in range(N_CORES):
                uc = buf[c * U : (c + 1) * U]
                valid = uc >= 0
                sv = np.zeros(U, np.int64)
                sv[valid] = ss[uc[valid]] - (SPLIT if s_hi else 0)
                sidx_parts[c].append(sv.astype(np.int16))
                dvals = np.zeros(U * K, np.int64)
                pvals = np.zeros(U * K, np.int64)
                ids = np.full(U * K, -1, np.int64)
                uu = np.arange(U)
                for m in range(K):
                    pos = (K * (uu // P) + m) * P + uu % P
                    dv = sd[uc[valid] + m]
                    dvals[pos[valid]] = dv >> 1
                    pvals[pos[valid]] = dv & 1
                    ids[pos[valid]] = order0[uc[valid] + m]
                didx_parts[c].append(dvals.astype(np.int16))
                par_parts[c].append(pvals)
                seq_parts[c].append(ids)
            # chunks
            o, rem = 0, U
            Gn = G_MAP[K]
            while rem > 0:
                n = min(Gn, rem)
                schedule.append((K, s_hi, u_off + o, e_off + o * K, n))
                o += n
                rem -= n
            u_off += U
            e_off += U * K

    seqs = np.stack([np.concatenate(p) for p in seq_parts])
    pars = np.stack([np.concatenate(p) for p in par_parts])
    sidx = [np.concatenate(p) for p in sidx_parts]
    didx = [np.concatenate(p) for p in didx_parts]
    return schedule, seqs, pars, sidx, didx, u_off, e_off


def _build_nc(schedule, u_total, e_total):
    SCOLS = u_total // 16
    DCOLS = e_total // 16
    TILES2 = 2 * (e_total // P)

    nc = bacc.Bacc(
        get_trn_type() or "TRN2",
        debug=False,
        dynamic_dma_scratch_size=32768,
        num_swdge_queues=NQ,
    )
    h = nc.dram_tensor("h", [NPAD, D], mybir.dt.float32, kind="ExternalInput")
    sidx = nc.dram_tensor("sidx", [P, SCOLS], mybir.dt.int16, kind="ExternalInput")
    didx = nc.dram_tensor("didx", [P, DCOLS], mybir.dt.int16, kind="ExternalInput")
    out = nc.dram_tensor("out", [P, TILES2], mybir.dt.float32, kind="ExternalOutput")

    h_ap = h[:]
    # dst side reads 512B node-row pairs: addr = idx * 512B
    h2 = AP(h_ap.tensor, 0, [[2 * D, NPAD // 2], [1, 2 * D]])
    # src side reads single 256B rows with per-range bases
    h_lo = h[0:SPLIT, :]
    h_hi = h[SPLIT:NPAD, :]
    nch = len(schedule)

    # greedy queue assignment balancing descriptor counts + fixed cost
    qloads = [0] * NQ
    qassign = []
    for (K, s_hi, uo, eo, n) in schedule:
        qs = min(range(NQ), key=lambda q: qloads[q])
        qloads[qs] += n + QFIX
        qd = min(range(NQ), key=lambda q: qloads[q])
        qloads[qd] += n * K + QFIX
        qassign.append((qs, qd))

    with ExitStack() as stack:
        ent = stack.enter_context
        hu = [ent(nc.sbuf_tensor(f"hu{i}", [P, 1024], mybir.dt.float32)) for i in range(NB)]
        hv = [ent(nc.sbuf_tensor(f"hv{i}", [P, 4096], mybir.dt.float32)) for i in range(NB)]
        sidx_sb = ent(nc.sbuf_tensor("sidx_sb", [P, SCOLS], mybir.dt.int16))
        didx_sb = ent(nc.sbuf_tensor("didx_sb", [P, DCOLS], mybir.dt.int16))
        outb = ent(nc.sbuf_tensor("outb", [P, TILES2], mybir.dt.float32))
        io = ent(nc.semaphore("io"))
        io2 = ent(nc.semaphore("io2"))
        gsem = [ent(nc.semaphore(f"g{i}")) for i in range(NB)]
        vsem = [ent(nc.semaphore(f"v{i}")) for i in range(NB)]
        mr = ent(nc.semaphore("mr"))

        def hu_ap(b, t_u):
            base = hu[b][:]
            return AP(base.tensor, 0, [[1024, P], [D, t_u], [1, D]])

        def hu_bcast(b, t_u, K):
            base = hu[b][:]
            return AP(base.tensor, 0, [[1024, P], [D, t_u], [0, 2 * K], [1, D]])

        def hv_gather_ap(b, t_e):
            base = hv[b][:]
            return AP(base.tensor, 0, [[4096, P], [2 * D, t_e], [1, 2 * D]])

        def hv_prod(b, t_u, K):
            base = hv[b][:]
            return AP(base.tensor, 0, [[4096, P], [2 * D * K, t_u], [D, 2 * K], [1, D]])

        with nc.Block() as block:

            @block.sync
            def _(sync):
                sync.dma_start(sidx_sb[:], sidx[:]).then_inc(io, 16)
                sync.dma_start(didx_sb[:], didx[:]).then_inc(io, 16)
                for b in range(NB):
                    uses = (nch - b + NB - 1) // NB
                    if uses:
                        sync.wait_ge(vsem[b], uses)
                sync.dma_start(out[:], outb[:]).then_inc(io2, 16)
                sync.wait_ge(io2, 16)

            @block.gpsimd
            def _(gp):
                gp.load_library(library_config.mlp)
                gp.wait_ge(io, 32)
                for c, (K, s_hi, uo, eo, n) in enumerate(schedule):
                    b = c % NB
                    qs, qd = qassign[c]
                    if c >= NB:
                        gp.wait_ge(vsem[b], c // NB)
                    t_u = n // P
                    gp.dma_gather(
                        hu_ap(b, t_u),
                        h_hi if s_hi else h_lo,
                        sidx_sb[:, uo // 16 : (uo + n) // 16],
                        n,
                        n,
                        D,
                        single_packet=False,
                        queue_num=qs,
                    ).then_inc(gsem[b], 16)
                    ne = n * K
                    gp.dma_gather(
                        hv_gather_ap(b, ne // P),
                        h2,
                        didx_sb[:, eo // 16 : (eo + ne) // 16],
                        ne,
                        ne,
                        2 * D,
                        single_packet=False,
                        queue_num=qd,
                    ).then_inc(gsem[b], 16)

            @block.vector
            def _(ve):
                for c, (K, s_hi, uo, eo, n) in enumerate(schedule):
                    b = c % NB
                    ve.wait_ge(gsem[b], 32 * (c // NB + 1))
                    t_u = n // P
                    prod = hv_prod(b, t_u, K)
                    ve.tensor_tensor(
                        out=prod, in0=prod, in1=hu_bcast(b, t_u, K),
                        op=mybir.AluOpType.mult,
                    ).then_inc(mr, 1)
                    ve.wait_ge(mr, c + 1)
                    ve.tensor_reduce(
                        out=outb[:, 2 * eo // P : 2 * eo // P + 2 * t_u * K],
                        in_=prod,
                        axis=mybir.AxisListType.X,
                        op=mybir.AluOpType.add,
                    ).then_inc(vsem[b], 1)

    nc.compile()
    return nc


def kernel(h, src, dst):
    global LAST_RESULT
    h = np.asarray(h, dtype=np.float32)
    hp = np.zeros((NPAD, D), np.float32)
    hp[:N_NODES] = h
    src = np.asarray(src).astype(np.int64)
    dst = np.asarray(dst).astype(np.int64)
    E = src.shape[0]

    schedule, seqs, pars, sidx, didx, u_total, e_total = _host_prep(src, dst)
    in_maps = [
        {"h": hp, "sidx": _wrap_idx(sidx[c]), "didx": _wrap_idx(didx[c])}
        for c in range(N_CORES)
    ]
    nc = _build_nc(schedule, u_total, e_total)

    if TRACE or os.environ.get("BASS_TRACE"):
        _ensure_ntff_hook()
    res = run_bass_kernel_spmd(nc, in_maps, core_ids=list(range(N_CORES)), trace=TRACE)
    LAST_RESULT = res

    out = np.empty(E, np.float32)
    i = np.arange(e_total)
    for c in range(N_CORES):
        dotq = res.results[c]["out"].T.reshape(-1)
        seq = seqs[c]
        valid = seq >= 0
        j = (2 * (i // P) + pars[c]) * P + (i % P)
        out[seq[valid]] = dotq[j[valid]]
    return out


# revision 13
# speedup vs baseline: 1.1498x; 1.1498x over previous
"""Trainium2 Bass kernel for per-edge dot products (GNN DotPredictor).

out[e] = sum(h[src[e]] * h[dst[e]]); 800k edges, h [50k, 64] f32, 8 cores.

Design (v3):
  - Edges sharded 8 ways; h replicated. Per-edge rows fetched from HBM with
    the Q7 `dma_gather` path. The Q7 descriptor generation (~8ns/descriptor
    per cpu pair) is the bottleneck, so it is parallelized 4x across the 4
    SWDGE queues (each queue's descriptors are generated by its own Q7 cpu
    pair) and minimized: edges are sorted by (range-group, src) and equal-src
    runs are decomposed into K-edge units (K in {8,4,2,1}); one 256B src
    descriptor serves K edges (hu broadcast via step-0 AP). dst side stays
    one 256B descriptor per edge.
  - int16 gather indices => 4-way range bucketing (src>=32768, dst>=32768)
    with per-range base pointers; host permutes edges, unpermutes results.
  - DVE: hu broadcast across K members via step-0 AP, in-place multiply
    into the hv tile, segment-reduce 64-feature dim to one score per edge.
  - Output [128, tiles] stored contiguously; host transposes + scatters.
"""

import os
from contextlib import ExitStack

import numpy as np

import concourse.bacc as bacc
import concourse.mybir as mybir
from concourse import library_config
from concourse.bass import AP
from concourse._compat import get_trn_type
from concourse.bass_utils import run_bass_kernel_spmd

N_NODES = 50000
NPAD = 50008  # h padded so reads past the last node stay in bounds
D = 64
P = 128
N_CORES = 8
SPLIT = 32768
NQ = 4  # SWDGE queues (each with its own Q7 descriptor-gen cpu pair)
NB = 10  # buffer pairs

G_MAP = {8: 512, 4: 1024, 2: 2048, 1: 2048}  # units per chunk

TRACE = False
LAST_RESULT = None


def _ensure_ntff_hook():
    """bass_utils' trace path imports antenv.axon_hooks, which this image's
    antenv package lacks. Recreate it from the boot helper so trace=True
    works; harmless no-op if the real module exists."""
    import sys
    import types

    try:
        import antenv.axon_hooks  # noqa: F401

        return
    except ImportError:
        pass
    try:
        import antenv
        from trn_agent_boot.trn_boot import _ntff_profile_via_ctypes

        hook = _ntff_profile_via_ctypes("/opt/axon/libaxon_pjrt.so")
        m = types.ModuleType("antenv.axon_hooks")
        m.get_axon_ntff_profile_hook = lambda: hook
        m.set_axon_ntff_profile_hook = lambda h: None
        sys.modules["antenv.axon_hooks"] = m
        antenv.axon_hooks = m
    except Exception:
        pass


def _wrap_idx(vals):
    """int16 index array [Npc] -> the [128, Npc/16] SBUF layout dma_gather
    expects (idx i at partition i%16, column i//16, replicated over the 8
    groups of 16 partitions — each SWDGE queue's Q7 pair reads its own
    group)."""
    w = vals.reshape(-1, 16).T  # [16, Npc/16]
    return np.ascontiguousarray(np.tile(w, (8, 1)))  # [128, Npc/16]


def _host_prep(src, dst):
    """Sort by (range-group, src); decompose equal-src runs into K-units.

    Returns (schedule, seqs, sidx_per_core, didx_per_core, u_total, e_total):
      schedule: list of (K, s_hi, d_hi, u_off, e_off, n_units), same all cores
      seqs: [N_CORES, e_total] global edge id per output position (-1 pad)
    """
    E = src.shape[0]
    g = (src >= SPLIT).astype(np.int8) * 2 + (dst >= SPLIT).astype(np.int8)
    order0 = np.lexsort((src, g))
    sg, ss, sd = g[order0], src[order0], dst[order0]

    new = np.ones(E, bool)
    new[1:] = (sg[1:] != sg[:-1]) | (ss[1:] != ss[:-1])
    run_start = np.flatnonzero(new)
    d = np.diff(np.append(run_start, E))
    run_id = np.cumsum(new) - 1
    r = np.arange(E) - run_start[run_id]
    dd = d[run_id]
    n8 = (dd // 8) * 8
    n4 = n8 + (((dd - n8) // 4) * 4)
    n2 = n4 + (((dd - n4) // 2) * 2)
    K_e = np.where(r < n8, 8, np.where(r < n4, 4, np.where(r < n2, 2, 1)))
    m_e = np.where(
        K_e == 8, r % 8,
        np.where(K_e == 4, (r - n8) % 4, np.where(K_e == 2, (r - n4) % 2, 0)),
    )
    first = m_e == 0

    pad_units = N_CORES * P
    schedule = []
    sidx_parts = [[] for _ in range(N_CORES)]
    didx_parts = [[] for _ in range(N_CORES)]
    seq_parts = [[] for _ in range(N_CORES)]
    u_off = 0
    e_off = 0
    for K in (8, 4, 2, 1):
        for gg in range(4):
            starts = np.flatnonzero(first & (K_e == K) & (sg == gg))
            if starts.size == 0:
                continue
            Upad = -(-starts.size // pad_units) * pad_units
            buf = np.full(Upad, -1, dtype=np.int64)
            buf[: starts.size] = starts
            U = Upad // N_CORES  # per-core units, multiple of 128
            s_hi, d_hi = gg >= 2, gg % 2 == 1
            for c in range(N_CORES):
                uc = buf[c * U : (c + 1) * U]
                valid = uc >= 0
                sv = np.zeros(U, np.int64)
                sv[valid] = ss[uc[valid]] - (SPLIT if s_hi else 0)
                sidx_parts[c].append(sv.astype(np.int16))
                dvals = np.zeros(U * K, np.int64)
                ids = np.full(U * K, -1, np.int64)
                uu = np.arange(U)
                for m in range(K):
                    pos = (K * (uu // P) + m) * P + uu % P
                    dvals[pos[valid]] = sd[uc[valid] + m] - (
                        SPLIT if d_hi else 0
                    )
                    ids[pos[valid]] = order0[uc[valid] + m]
                didx_parts[c].append(dvals.astype(np.int16))
                seq_parts[c].append(ids)
            # chunks
            o, rem = 0, U
            Gn = G_MAP[K]
            while rem > 0:
                n = min(Gn, rem)
                schedule.append((K, s_hi, d_hi, u_off + o, e_off + o * K, n))
                o += n
                rem -= n
            u_off += U
            e_off += U * K

    seqs = np.stack([np.concatenate(p) for p in seq_parts])
    sidx = [np.concatenate(p) for p in sidx_parts]
    didx = [np.concatenate(p) for p in didx_parts]
    return schedule, seqs, sidx, didx, u_off, e_off


def _build_nc(schedule, u_total, e_total):
    SCOLS = u_total // 16
    DCOLS = e_total // 16
    TILES = e_total // P

    nc = bacc.Bacc(
        get_trn_type() or "TRN2",
        debug=False,
        dynamic_dma_scratch_size=32768,
        num_swdge_queues=NQ,
    )
    h = nc.dram_tensor("h", [NPAD, D], mybir.dt.float32, kind="ExternalInput")
    sidx = nc.dram_tensor("sidx", [P, SCOLS], mybir.dt.int16, kind="ExternalInput")
    didx = nc.dram_tensor("didx", [P, DCOLS], mybir.dt.int16, kind="ExternalInput")
    out = nc.dram_tensor("out", [P, TILES], mybir.dt.float32, kind="ExternalOutput")

    # per-row base pointers for the two int16 index ranges
    h_lo = h[0:SPLIT, :]
    h_hi = h[SPLIT:NPAD, :]
    nch = len(schedule)

    # greedy queue assignment balancing descriptor counts plus per-instruction
    # fixed cost; (queue, order) per chunk for (src_gather, dst_gather)
    QFIX = 112  # fixed cost per gather in descriptor-equivalents (~0.9us)
    qloads = [0] * NQ
    qassign = []
    for (K, s_hi, d_hi, uo, eo, n) in schedule:
        qs = min(range(NQ), key=lambda q: qloads[q])
        qloads[qs] += n + QFIX
        qd = min(range(NQ), key=lambda q: qloads[q])
        qloads[qd] += n * K + QFIX
        qassign.append((qs, qd))

    # split the idx loads: slice A covers the first NB chunks so gathers can
    # start before the whole idx tiles land; slice B covers the rest
    if nch > NB:
        uoB = schedule[NB][3]
        eoB = schedule[NB][4]
    else:
        uoB, eoB = u_total, e_total

    # chunked output: one store per buffer round
    rounds = []
    for r in range(-(-nch // NB)):
        c_lo, c_hi = r * NB, min(nch, (r + 1) * NB)
        eo_lo = schedule[c_lo][4]
        eo_hi = schedule[c_hi][4] if c_hi < nch else e_total
        rounds.append((c_lo, c_hi, eo_lo, eo_hi))

    with ExitStack() as stack:
        ent = stack.enter_context
        hu = [ent(nc.sbuf_tensor(f"hu{i}", [P, 1024], mybir.dt.float32)) for i in range(NB)]
        hv = [ent(nc.sbuf_tensor(f"hv{i}", [P, 2048], mybir.dt.float32)) for i in range(NB)]
        sidx_sb = ent(nc.sbuf_tensor("sidx_sb", [P, SCOLS], mybir.dt.int16))
        didx_sb = ent(nc.sbuf_tensor("didx_sb", [P, DCOLS], mybir.dt.int16))
        outb = ent(nc.sbuf_tensor("outb", [P, TILES], mybir.dt.float32))
        io = ent(nc.semaphore("io"))
        iob = ent(nc.semaphore("iob"))
        io2 = ent(nc.semaphore("io2"))
        gsem = [ent(nc.semaphore(f"g{i}")) for i in range(NB)]
        vsem = [ent(nc.semaphore(f"v{i}")) for i in range(NB)]
        mr = ent(nc.semaphore("mr"))

        def hu_ap(b, t_u):
            base = hu[b][:]
            return AP(base.tensor, 0, [[1024, P], [D, t_u], [1, D]])

        def hu_bcast(b, t_u, K):
            base = hu[b][:]
            return AP(base.tensor, 0, [[1024, P], [D, t_u], [0, K], [1, D]])

        def hv_ap(b, t_e):
            base = hv[b][:]
            return AP(base.tensor, 0, [[2048, P], [D, t_e], [1, D]])

        def hv_4d(b, t_u, K):
            base = hv[b][:]
            return AP(base.tensor, 0, [[2048, P], [D * K, t_u], [D, K], [1, D]])

        with nc.Block() as block:

            @block.sync
            def _(sync):
                sync.dma_start(sidx_sb[:, : uoB // 16], sidx[:, : uoB // 16]).then_inc(io, 16)
                sync.dma_start(didx_sb[:, : eoB // 16], didx[:, : eoB // 16]).then_inc(io, 16)
                if uoB < u_total:
                    sync.dma_start(sidx_sb[:, uoB // 16 :], sidx[:, uoB // 16 :]).then_inc(iob, 16)
                if eoB < e_total:
                    sync.dma_start(didx_sb[:, eoB // 16 :], didx[:, eoB // 16 :]).then_inc(iob, 16)
                for r, (c_lo, c_hi, eo_lo, eo_hi) in enumerate(rounds):
                    for c in range(c_lo, c_hi):
                        sync.wait_ge(vsem[c % NB], c // NB + 1)
                    sync.dma_start(
                        out[:, eo_lo // P : eo_hi // P],
                        outb[:, eo_lo // P : eo_hi // P],
                    ).then_inc(io2, 16)
                sync.wait_ge(io2, 16 * len(rounds))

            @block.gpsimd
            def _(gp):
                gp.load_library(library_config.mlp)
                gp.wait_ge(io, 32)
                for c, (K, s_hi, d_hi, uo, eo, n) in enumerate(schedule):
                    b = c % NB
                    qs, qd = qassign[c]
                    if c == NB and (uoB < u_total or eoB < e_total):
                        gp.wait_ge(iob, 16 * ((uoB < u_total) + (eoB < e_total)))
                    if c >= NB:
                        gp.wait_ge(vsem[b], c // NB)
                    t_u = n // P
                    gp.dma_gather(
                        hu_ap(b, t_u),
                        h_hi if s_hi else h_lo,
                        sidx_sb[:, uo // 16 : (uo + n) // 16],
                        n,
                        n,
                        D,
                        single_packet=False,
                        queue_num=qs,
                    ).then_inc(gsem[b], 16)
                    ne = n * K
                    gp.dma_gather(
                        hv_ap(b, ne // P),
                        h_hi if d_hi else h_lo,
                        didx_sb[:, eo // 16 : (eo + ne) // 16],
                        ne,
                        ne,
                        D,
                        single_packet=False,
                        queue_num=qd,
                    ).then_inc(gsem[b], 16)

            @block.vector
            def _(ve):
                for c, (K, s_hi, d_hi, uo, eo, n) in enumerate(schedule):
                    b = c % NB
                    ve.wait_ge(gsem[b], 32 * (c // NB + 1))
                    t_u = n // P
                    t_e = t_u * K
                    if K == 1:
                        prod_in1 = hu_ap(b, t_u)
                        prod = hv_ap(b, t_e)
                    else:
                        prod_in1 = hu_bcast(b, t_u, K)
                        prod = hv_4d(b, t_u, K)
                    ve.tensor_tensor(
                        out=prod, in0=prod, in1=prod_in1,
                        op=mybir.AluOpType.mult,
                    ).then_inc(mr, 1)
                    ve.wait_ge(mr, c + 1)
                    ve.tensor_reduce(
                        out=outb[:, eo // P : eo // P + t_e],
                        in_=prod,
                        axis=mybir.AxisListType.X,
                        op=mybir.AluOpType.add,
                    ).then_inc(vsem[b], 1)

    nc.compile()
    return nc


def kernel(h, src, dst):
    global LAST_RESULT
    h = np.asarray(h, dtype=np.float32)
    hp = np.zeros((NPAD, D), np.float32)
    hp[:N_NODES] = h
    src = np.asarray(src).astype(np.int64)
    dst = np.asarray(dst).astype(np.int64)
    E = src.shape[0]

    schedule, seqs, sidx, didx, u_total, e_total = _host_prep(src, dst)
    in_maps = [
        {"h": hp, "sidx": _wrap_idx(sidx[c]), "didx": _wrap_idx(didx[c])}
        for c in range(N_CORES)
    ]
    nc = _build_nc(schedule, u_total, e_total)

    if TRACE or os.environ.get("BASS_TRACE"):
        _ensure_ntff_hook()
    res = run_bass_kernel_spmd(nc, in_maps, core_ids=list(range(N_CORES)), trace=TRACE)
    LAST_RESULT = res

    out = np.empty(E, np.float32)
    for c in range(N_CORES):
        dots = res.results[c]["out"].T.reshape(-1)
        seq = seqs[c]
        valid = seq >= 0
        out[seq[valid]] = dots[valid]
    return out


# revision 14
# speedup vs baseline: 1.1505x; 1.0005x over previous
"""Trainium2 Bass kernel for per-edge dot products (GNN DotPredictor).

out[e] = sum(h[src[e]] * h[dst[e]]); 800k edges, h [50k, 64] f32, 8 cores.

Design (v3):
  - Edges sharded 8 ways; h replicated. Per-edge rows fetched from HBM with
    the Q7 `dma_gather` path. The Q7 descriptor generation (~8ns/descriptor
    per cpu pair) is the bottleneck, so it is parallelized 4x across the 4
    SWDGE queues (each queue's descriptors are generated by its own Q7 cpu
    pair) and minimized: edges are sorted by (range-group, src) and equal-src
    runs are decomposed into K-edge units (K in {8,4,2,1}); one 256B src
    descriptor serves K edges (hu broadcast via step-0 AP). dst side stays
    one 256B descriptor per edge.
  - int16 gather indices => 4-way range bucketing (src>=32768, dst>=32768)
    with per-range base pointers; host permutes edges, unpermutes results.
  - DVE: hu broadcast across K members via step-0 AP, in-place multiply
    into the hv tile, segment-reduce 64-feature dim to one score per edge.
  - Output [128, tiles] stored contiguously; host transposes + scatters.
"""

import os
from contextlib import ExitStack

import numpy as np

import concourse.bacc as bacc
import concourse.mybir as mybir
from concourse import library_config
from concourse.bass import AP
from concourse._compat import get_trn_type
from concourse.bass_utils import run_bass_kernel_spmd

N_NODES = 50000
NPAD = 50008  # h padded so reads past the last node stay in bounds
D = 64
P = 128
N_CORES = 8
SPLIT = 32768
NQ = 4  # SWDGE queues (each with its own Q7 descriptor-gen cpu pair)
NB = 10  # buffer pairs

G_MAP = {8: 512, 4: 1024, 2: 2048, 1: 2048}  # units per chunk

TRACE = False
LAST_RESULT = None


def _ensure_ntff_hook():
    """bass_utils' trace path imports antenv.axon_hooks, which this image's
    antenv package lacks. Recreate it from the boot helper so trace=True
    works; harmless no-op if the real module exists."""
    import sys
    import types

    try:
        import antenv.axon_hooks  # noqa: F401

        return
    except ImportError:
        pass
    try:
        import antenv
        from trn_agent_boot.trn_boot import _ntff_profile_via_ctypes

        hook = _ntff_profile_via_ctypes("/opt/axon/libaxon_pjrt.so")
        m = types.ModuleType("antenv.axon_hooks")
        m.get_axon_ntff_profile_hook = lambda: hook
        m.set_axon_ntff_profile_hook = lambda h: None
        sys.modules["antenv.axon_hooks"] = m
        antenv.axon_hooks = m
    except Exception:
        pass


def _wrap_idx(vals):
    """int16 index array [Npc] -> the [128, Npc/16] SBUF layout dma_gather
    expects (idx i at partition i%16, column i//16, replicated over the 8
    groups of 16 partitions — each SWDGE queue's Q7 pair reads its own
    group)."""
    w = vals.reshape(-1, 16).T  # [16, Npc/16]
    return np.ascontiguousarray(np.tile(w, (8, 1)))  # [128, Npc/16]


def _host_prep(src, dst):
    """Sort by (range-group, src); decompose equal-src runs into K-units.

    Returns (schedule, seqs, sidx_per_core, didx_per_core, u_total, e_total):
      schedule: list of (K, s_hi, d_hi, u_off, e_off, n_units), same all cores
      seqs: [N_CORES, e_total] global edge id per output position (-1 pad)
    """
    E = src.shape[0]
    g = (src >= SPLIT).astype(np.int8) * 2 + (dst >= SPLIT).astype(np.int8)
    order0 = np.lexsort((src, g))
    sg, ss, sd = g[order0], src[order0], dst[order0]

    new = np.ones(E, bool)
    new[1:] = (sg[1:] != sg[:-1]) | (ss[1:] != ss[:-1])
    run_start = np.flatnonzero(new)
    d = np.diff(np.append(run_start, E))
    run_id = np.cumsum(new) - 1
    r = np.arange(E) - run_start[run_id]
    dd = d[run_id]
    n8 = (dd // 8) * 8
    n4 = n8 + (((dd - n8) // 4) * 4)
    n2 = n4 + (((dd - n4) // 2) * 2)
    K_e = np.where(r < n8, 8, np.where(r < n4, 4, np.where(r < n2, 2, 1)))
    m_e = np.where(
        K_e == 8, r % 8,
        np.where(K_e == 4, (r - n8) % 4, np.where(K_e == 2, (r - n4) % 2, 0)),
    )
    first = m_e == 0

    pad_units = N_CORES * P
    schedule = []
    sidx_parts = [[] for _ in range(N_CORES)]
    didx_parts = [[] for _ in range(N_CORES)]
    seq_parts = [[] for _ in range(N_CORES)]
    u_off = 0
    e_off = 0
    for K in (8, 4, 2, 1):
        for gg in range(4):
            starts = np.flatnonzero(first & (K_e == K) & (sg == gg))
            if starts.size == 0:
                continue
            Upad = -(-starts.size // pad_units) * pad_units
            buf = np.full(Upad, -1, dtype=np.int64)
            buf[: starts.size] = starts
            U = Upad // N_CORES  # per-core units, multiple of 128
            s_hi, d_hi = gg >= 2, gg % 2 == 1
            for c in range(N_CORES):
                uc = buf[c * U : (c + 1) * U]
                valid = uc >= 0
                sv = np.zeros(U, np.int64)
                sv[valid] = ss[uc[valid]] - (SPLIT if s_hi else 0)
                sidx_parts[c].append(sv.astype(np.int16))
                dvals = np.zeros(U * K, np.int64)
                ids = np.full(U * K, -1, np.int64)
                uu = np.arange(U)
                for m in range(K):
                    pos = (K * (uu // P) + m) * P + uu % P
                    dvals[pos[valid]] = sd[uc[valid] + m] - (
                        SPLIT if d_hi else 0
                    )
                    ids[pos[valid]] = order0[uc[valid] + m]
                didx_parts[c].append(dvals.astype(np.int16))
                seq_parts[c].append(ids)
            # chunks
            o, rem = 0, U
            Gn = G_MAP[K]
            while rem > 0:
                n = min(Gn, rem)
                schedule.append((K, s_hi, d_hi, u_off + o, e_off + o * K, n))
                o += n
                rem -= n
            u_off += U
            e_off += U * K

    seqs = np.stack([np.concatenate(p) for p in seq_parts])
    sidx = [np.concatenate(p) for p in sidx_parts]
    didx = [np.concatenate(p) for p in didx_parts]
    return schedule, seqs, sidx, didx, u_off, e_off


def _build_nc(schedule, u_total, e_total):
    SCOLS = u_total // 16
    DCOLS = e_total // 16
    TILES = e_total // P

    nc = bacc.Bacc(
        get_trn_type() or "TRN2",
        debug=False,
        dynamic_dma_scratch_size=32768,
        num_swdge_queues=NQ,
    )
    h = nc.dram_tensor("h", [NPAD, D], mybir.dt.float32, kind="ExternalInput")
    sidx = nc.dram_tensor("sidx", [P, SCOLS], mybir.dt.int16, kind="ExternalInput")
    didx = nc.dram_tensor("didx", [P, DCOLS], mybir.dt.int16, kind="ExternalInput")
    out = nc.dram_tensor("out", [P, TILES], mybir.dt.float32, kind="ExternalOutput")

    # per-row base pointers for the two int16 index ranges
    h_lo = h[0:SPLIT, :]
    h_hi = h[SPLIT:NPAD, :]
    nch = len(schedule)

    # greedy queue assignment balancing descriptor counts plus per-instruction
    # fixed cost; (queue, order) per chunk for (src_gather, dst_gather)
    QFIX = 112  # fixed cost per gather in descriptor-equivalents (~0.9us)
    qloads = [0] * NQ
    qassign = []
    for (K, s_hi, d_hi, uo, eo, n) in schedule:
        qs = min(range(NQ), key=lambda q: qloads[q])
        qloads[qs] += n + QFIX
        qd = min(range(NQ), key=lambda q: qloads[q])
        qloads[qd] += n * K + QFIX
        qassign.append((qs, qd))

    # split the idx loads: slice A covers the first NB chunks so gathers can
    # start before the whole idx tiles land; slice B covers the rest
    if nch > NB:
        uoB = schedule[NB][3]
        eoB = schedule[NB][4]
    else:
        uoB, eoB = u_total, e_total

    # chunked output: one store per buffer round
    rounds = []
    for r in range(-(-nch // NB)):
        c_lo, c_hi = r * NB, min(nch, (r + 1) * NB)
        eo_lo = schedule[c_lo][4]
        eo_hi = schedule[c_hi][4] if c_hi < nch else e_total
        rounds.append((c_lo, c_hi, eo_lo, eo_hi))

    with ExitStack() as stack:
        ent = stack.enter_context
        hu = [ent(nc.sbuf_tensor(f"hu{i}", [P, 1024], mybir.dt.float32)) for i in range(NB)]
        hv = [ent(nc.sbuf_tensor(f"hv{i}", [P, 2048], mybir.dt.float32)) for i in range(NB)]
        sidx_sb = ent(nc.sbuf_tensor("sidx_sb", [P, SCOLS], mybir.dt.int16))
        didx_sb = ent(nc.sbuf_tensor("didx_sb", [P, DCOLS], mybir.dt.int16))
        outb = ent(nc.sbuf_tensor("outb", [P, TILES], mybir.dt.float32))
        io = ent(nc.semaphore("io"))
        iob = ent(nc.semaphore("iob"))
        io2 = ent(nc.semaphore("io2"))
        gsem = [ent(nc.semaphore(f"g{i}")) for i in range(NB)]
        vsem = [ent(nc.semaphore(f"v{i}")) for i in range(NB)]
        mr = ent(nc.semaphore("mr"))

        def hu_ap(b, t_u):
            base = hu[b][:]
            return AP(base.tensor, 0, [[1024, P], [D, t_u], [1, D]])

        def hu_bcast(b, t_u, K):
            base = hu[b][:]
            return AP(base.tensor, 0, [[1024, P], [D, t_u], [0, K], [1, D]])

        def hv_ap(b, t_e):
            base = hv[b][:]
            return AP(base.tensor, 0, [[2048, P], [D, t_e], [1, D]])

        def hv_4d(b, t_u, K):
            base = hv[b][:]
            return AP(base.tensor, 0, [[2048, P], [D * K, t_u], [D, K], [1, D]])

        with nc.Block() as block:

            @block.sync
            def _(sync):
                sync.dma_start(sidx_sb[:, : uoB // 16], sidx[:, : uoB // 16]).then_inc(io, 16)
                sync.dma_start(didx_sb[:, : eoB // 16], didx[:, : eoB // 16]).then_inc(io, 16)
                if uoB < u_total:
                    sync.dma_start(sidx_sb[:, uoB // 16 :], sidx[:, uoB // 16 :]).then_inc(iob, 16)
                if eoB < e_total:
                    sync.dma_start(didx_sb[:, eoB // 16 :], didx[:, eoB // 16 :]).then_inc(iob, 16)
                for r, (c_lo, c_hi, eo_lo, eo_hi) in enumerate(rounds):
                    for c in range(c_lo, c_hi):
                        sync.wait_ge(vsem[c % NB], c // NB + 1)
                    sync.dma_start(
                        out[:, eo_lo // P : eo_hi // P],
                        outb[:, eo_lo // P : eo_hi // P],
                    ).then_inc(io2, 16)
                sync.wait_ge(io2, 16 * len(rounds))

            @block.gpsimd
            def _(gp):
                gp.load_library(library_config.mlp)
                # preload one register per distinct gather size so the hot
                # loop issues no per-gather MOVEs on the sequencer
                sizes = sorted({n for (_, _, _, _, _, n) in schedule}
                               | {n * K for (K, _, _, _, _, n) in schedule})
                nregs = {n: gp.to_reg(n) for n in sizes}
                gp.wait_ge(io, 32)
                for c, (K, s_hi, d_hi, uo, eo, n) in enumerate(schedule):
                    b = c % NB
                    qs, qd = qassign[c]
                    if c == NB and (uoB < u_total or eoB < e_total):
                        gp.wait_ge(iob, 16 * ((uoB < u_total) + (eoB < e_total)))
                    if c >= NB:
                        gp.wait_ge(vsem[b], c // NB)
                    t_u = n // P
                    gp.dma_gather(
                        hu_ap(b, t_u),
                        h_hi if s_hi else h_lo,
                        sidx_sb[:, uo // 16 : (uo + n) // 16],
                        n,
                        nregs[n],
                        D,
                        single_packet=False,
                        queue_num=qs,
                    ).then_inc(gsem[b], 16)
                    ne = n * K
                    gp.dma_gather(
                        hv_ap(b, ne // P),
                        h_hi if d_hi else h_lo,
                        didx_sb[:, eo // 16 : (eo + ne) // 16],
                        ne,
                        nregs[ne],
                        D,
                        single_packet=False,
                        queue_num=qd,
                    ).then_inc(gsem[b], 16)

            @block.vector
            def _(ve):
                for c, (K, s_hi, d_hi, uo, eo, n) in enumerate(schedule):
                    b = c % NB
                    ve.wait_ge(gsem[b], 32 * (c // NB + 1))
                    t_u = n // P
                    t_e = t_u * K
                    if K == 1:
                        prod_in1 = hu_ap(b, t_u)
                        prod = hv_ap(b, t_e)
                    else:
                        prod_in1 = hu_bcast(b, t_u, K)
                        prod = hv_4d(b, t_u, K)
                    ve.tensor_tensor(
                        out=prod, in0=prod, in1=prod_in1,
                        op=mybir.AluOpType.mult,
                    ).then_inc(mr, 1)
                    ve.wait_ge(mr, c + 1)
                    ve.tensor_reduce(
                        out=outb[:, eo // P : eo // P + t_e],
                        in_=prod,
                        axis=mybir.AxisListType.X,
                        op=mybir.AluOpType.add,
                    ).then_inc(vsem[b], 1)

    nc.compile()
    return nc


def kernel(h, src, dst):
    global LAST_RESULT
    h = np.asarray(h, dtype=np.float32)
    hp = np.zeros((NPAD, D), np.float32)
    hp[:N_NODES] = h
    src = np.asarray(src).astype(np.int64)
    dst = np.asarray(dst).astype(np.int64)
    E = src.shape[0]

    schedule, seqs, sidx, didx, u_total, e_total = _host_prep(src, dst)
    in_maps = [
        {"h": hp, "sidx": _wrap_idx(sidx[c]), "didx": _wrap_idx(didx[c])}
        for c in range(N_CORES)
    ]
    nc = _build_nc(schedule, u_total, e_total)

    if TRACE or os.environ.get("BASS_TRACE"):
        _ensure_ntff_hook()
    res = run_bass_kernel_spmd(nc, in_maps, core_ids=list(range(N_CORES)), trace=TRACE)
    LAST_RESULT = res

    out = np.empty(E, np.float32)
    for c in range(N_CORES):
        dots = res.results[c]["out"].T.reshape(-1)
        seq = seqs[c]
        valid = seq >= 0
        out[seq[valid]] = dots[valid]
    return out


# revision 15
# speedup vs baseline: 1.1576x; 1.0062x over previous
"""Trainium2 Bass kernel for per-edge dot products (GNN DotPredictor).

out[e] = sum(h[src[e]] * h[dst[e]]); 800k edges, h [50k, 64] f32, 8 cores.

Design (v3):
  - Edges sharded 8 ways; h replicated. Per-edge rows fetched from HBM with
    the Q7 `dma_gather` path. The Q7 descriptor generation (~8ns/descriptor
    per cpu pair) is the bottleneck, so it is parallelized 4x across the 4
    SWDGE queues (each queue's descriptors are generated by its own Q7 cpu
    pair) and minimized: edges are sorted by (range-group, src) and equal-src
    runs are decomposed into K-edge units (K in {8,4,2,1}); one 256B src
    descriptor serves K edges (hu broadcast via step-0 AP). dst side stays
    one 256B descriptor per edge.
  - int16 gather indices => 4-way range bucketing (src>=32768, dst>=32768)
    with per-range base pointers; host permutes edges, unpermutes results.
  - DVE: hu broadcast across K members via step-0 AP, in-place multiply
    into the hv tile, segment-reduce 64-feature dim to one score per edge.
  - Output [128, tiles] stored contiguously; host transposes + scatters.
"""

import os
from contextlib import ExitStack

import numpy as np

import concourse.bacc as bacc
import concourse.mybir as mybir
from concourse import library_config
from concourse.bass import AP
from concourse._compat import get_trn_type
from concourse.bass_utils import run_bass_kernel_spmd

N_NODES = 50000
NPAD = 50008  # h padded so reads past the last node stay in bounds
D = 64
P = 128
N_CORES = 8
SPLIT = 32768
NQ = 4  # SWDGE queues (each with its own Q7 descriptor-gen cpu pair)
NB = 10  # buffer pairs

G_MAP = {8: 512, 4: 1024, 2: 2048, 1: 2048}  # units per chunk

TRACE = False
LAST_RESULT = None


def _ensure_ntff_hook():
    """bass_utils' trace path imports antenv.axon_hooks, which this image's
    antenv package lacks. Recreate it from the boot helper so trace=True
    works; harmless no-op if the real module exists."""
    import sys
    import types

    try:
        import antenv.axon_hooks  # noqa: F401

        return
    except ImportError:
        pass
    try:
        import antenv
        from trn_agent_boot.trn_boot import _ntff_profile_via_ctypes

        hook = _ntff_profile_via_ctypes("/opt/axon/libaxon_pjrt.so")
        m = types.ModuleType("antenv.axon_hooks")
        m.get_axon_ntff_profile_hook = lambda: hook
        m.set_axon_ntff_profile_hook = lambda h: None
        sys.modules["antenv.axon_hooks"] = m
        antenv.axon_hooks = m
    except Exception:
        pass


def _wrap_idx(vals):
    """int16 index array [Npc] -> the [128, Npc/16] SBUF layout dma_gather
    expects (idx i at partition i%16, column i//16, replicated over the 8
    groups of 16 partitions — each SWDGE queue's Q7 pair reads its own
    group)."""
    w = vals.reshape(-1, 16).T  # [16, Npc/16]
    return np.ascontiguousarray(np.tile(w, (8, 1)))  # [128, Npc/16]


def _host_prep(src, dst):
    """Sort by (range-group, src); decompose equal-src runs into K-units.

    Returns (schedule, seqs, sidx_per_core, didx_per_core, u_total, e_total):
      schedule: list of (K, s_hi, d_hi, u_off, e_off, n_units), same all cores
      seqs: [N_CORES, e_total] global edge id per output position (-1 pad)
    """
    E = src.shape[0]
    g = (src >= SPLIT).astype(np.int8) * 2 + (dst >= SPLIT).astype(np.int8)
    order0 = np.lexsort((src, g))
    sg, ss, sd = g[order0], src[order0], dst[order0]

    new = np.ones(E, bool)
    new[1:] = (sg[1:] != sg[:-1]) | (ss[1:] != ss[:-1])
    run_start = np.flatnonzero(new)
    d = np.diff(np.append(run_start, E))
    run_id = np.cumsum(new) - 1
    r = np.arange(E) - run_start[run_id]
    dd = d[run_id]
    n8 = (dd // 8) * 8
    n4 = n8 + (((dd - n8) // 4) * 4)
    n2 = n4 + (((dd - n4) // 2) * 2)
    K_e = np.where(r < n8, 8, np.where(r < n4, 4, np.where(r < n2, 2, 1)))
    m_e = np.where(
        K_e == 8, r % 8,
        np.where(K_e == 4, (r - n8) % 4, np.where(K_e == 2, (r - n4) % 2, 0)),
    )
    first = m_e == 0

    pad_units = N_CORES * P
    schedule = []
    sidx_parts = [[] for _ in range(N_CORES)]
    didx_parts = [[] for _ in range(N_CORES)]
    seq_parts = [[] for _ in range(N_CORES)]
    u_off = 0
    e_off = 0
    for K in (8, 4, 2, 1):
        for gg in range(4):
            starts = np.flatnonzero(first & (K_e == K) & (sg == gg))
            if starts.size == 0:
                continue
            Upad = -(-starts.size // pad_units) * pad_units
            buf = np.full(Upad, -1, dtype=np.int64)
            buf[: starts.size] = starts
            U = Upad // N_CORES  # per-core units, multiple of 128
            s_hi, d_hi = gg >= 2, gg % 2 == 1
            for c in range(N_CORES):
                uc = buf[c * U : (c + 1) * U]
                valid = uc >= 0
                sv = np.zeros(U, np.int64)
                sv[valid] = ss[uc[valid]] - (SPLIT if s_hi else 0)
                sidx_parts[c].append(sv.astype(np.int16))
                dvals = np.zeros(U * K, np.int64)
                ids = np.full(U * K, -1, np.int64)
                uu = np.arange(U)
                for m in range(K):
                    pos = (K * (uu // P) + m) * P + uu % P
                    dvals[pos[valid]] = sd[uc[valid] + m] - (
                        SPLIT if d_hi else 0
                    )
                    ids[pos[valid]] = order0[uc[valid] + m]
                didx_parts[c].append(dvals.astype(np.int16))
                seq_parts[c].append(ids)
            # chunks
            o, rem = 0, U
            Gn = G_MAP[K]
            while rem > 0:
                n = min(Gn, rem)
                schedule.append((K, s_hi, d_hi, u_off + o, e_off + o * K, n))
                o += n
                rem -= n
            u_off += U
            e_off += U * K

    seqs = np.stack([np.concatenate(p) for p in seq_parts])
    sidx = [np.concatenate(p) for p in sidx_parts]
    didx = [np.concatenate(p) for p in didx_parts]
    return schedule, seqs, sidx, didx, u_off, e_off


def _build_nc(schedule, u_total, e_total):
    SCOLS = u_total // 16
    DCOLS = e_total // 16
    TILES = e_total // P

    nc = bacc.Bacc(
        get_trn_type() or "TRN2",
        debug=False,
        dynamic_dma_scratch_size=32768,
        num_swdge_queues=NQ,
    )
    h = nc.dram_tensor("h", [NPAD, D], mybir.dt.float32, kind="ExternalInput")
    sidx = nc.dram_tensor("sidx", [P, SCOLS], mybir.dt.int16, kind="ExternalInput")
    didx = nc.dram_tensor("didx", [P, DCOLS], mybir.dt.int16, kind="ExternalInput")
    out = nc.dram_tensor("out", [P, TILES], mybir.dt.float32, kind="ExternalOutput")

    # per-row base pointers for the two int16 index ranges
    h_lo = h[0:SPLIT, :]
    h_hi = h[SPLIT:NPAD, :]
    nch = len(schedule)

    # greedy queue assignment balancing descriptor counts; (queue, order) per
    # chunk for (src_gather, dst_gather)
    qloads = [0] * NQ
    qassign = []
    for (K, s_hi, d_hi, uo, eo, n) in schedule:
        qs = min(range(NQ), key=lambda q: qloads[q])
        qloads[qs] += n
        qd = min(range(NQ), key=lambda q: qloads[q])
        qloads[qd] += n * K
        qassign.append((qs, qd))

    with ExitStack() as stack:
        ent = stack.enter_context
        hu = [ent(nc.sbuf_tensor(f"hu{i}", [P, 1024], mybir.dt.float32)) for i in range(NB)]
        hv = [ent(nc.sbuf_tensor(f"hv{i}", [P, 2048], mybir.dt.float32)) for i in range(NB)]
        sidx_sb = ent(nc.sbuf_tensor("sidx_sb", [P, SCOLS], mybir.dt.int16))
        didx_sb = ent(nc.sbuf_tensor("didx_sb", [P, DCOLS], mybir.dt.int16))
        outb = ent(nc.sbuf_tensor("outb", [P, TILES], mybir.dt.float32))
        io = ent(nc.semaphore("io"))
        io2 = ent(nc.semaphore("io2"))
        gsem = [ent(nc.semaphore(f"g{i}")) for i in range(NB)]
        vsem = [ent(nc.semaphore(f"v{i}")) for i in range(NB)]
        mr = ent(nc.semaphore("mr"))

        def hu_ap(b, t_u):
            base = hu[b][:]
            return AP(base.tensor, 0, [[1024, P], [D, t_u], [1, D]])

        def hu_bcast(b, t_u, K):
            base = hu[b][:]
            return AP(base.tensor, 0, [[1024, P], [D, t_u], [0, K], [1, D]])

        def hv_ap(b, t_e):
            base = hv[b][:]
            return AP(base.tensor, 0, [[2048, P], [D, t_e], [1, D]])

        def hv_4d(b, t_u, K):
            base = hv[b][:]
            return AP(base.tensor, 0, [[2048, P], [D * K, t_u], [D, K], [1, D]])

        with nc.Block() as block:

            @block.sync
            def _(sync):
                sync.dma_start(sidx_sb[:], sidx[:]).then_inc(io, 16)
                sync.dma_start(didx_sb[:], didx[:]).then_inc(io, 16)
                for b in range(NB):
                    uses = (nch - b + NB - 1) // NB
                    if uses:
                        sync.wait_ge(vsem[b], uses)
                sync.dma_start(out[:], outb[:]).then_inc(io2, 16)
                sync.wait_ge(io2, 16)

            @block.gpsimd
            def _(gp):
                gp.load_library(library_config.mlp)
                gp.wait_ge(io, 32)
                for c, (K, s_hi, d_hi, uo, eo, n) in enumerate(schedule):
                    b = c % NB
                    qs, qd = qassign[c]
                    if c >= NB:
                        gp.wait_ge(vsem[b], c // NB)
                    t_u = n // P
                    gp.dma_gather(
                        hu_ap(b, t_u),
                        h_hi if s_hi else h_lo,
                        sidx_sb[:, uo // 16 : (uo + n) // 16],
                        n,
                        n,
                        D,
                        single_packet=False,
                        queue_num=qs,
                    ).then_inc(gsem[b], 16)
                    ne = n * K
                    gp.dma_gather(
                        hv_ap(b, ne // P),
                        h_hi if d_hi else h_lo,
                        didx_sb[:, eo // 16 : (eo + ne) // 16],
                        ne,
                        ne,
                        D,
                        single_packet=False,
                        queue_num=qd,
                    ).then_inc(gsem[b], 16)

            @block.vector
            def _(ve):
                for c, (K, s_hi, d_hi, uo, eo, n) in enumerate(schedule):
                    b = c % NB
                    ve.wait_ge(gsem[b], 32 * (c // NB + 1))
                    t_u = n // P
                    t_e = t_u * K
                    if K == 1:
                        prod_in1 = hu_ap(b, t_u)
                        prod = hv_ap(b, t_e)
                    else:
                        prod_in1 = hu_bcast(b, t_u, K)
                        prod = hv_4d(b, t_u, K)
                    ve.tensor_tensor(
                        out=prod, in0=prod, in1=prod_in1,
                        op=mybir.AluOpType.mult,
                    ).then_inc(mr, 1)
                    ve.wait_ge(mr, c + 1)
                    ve.tensor_reduce(
                        out=outb[:, eo // P : eo // P + t_e],
                        in_=prod,
                        axis=mybir.AxisListType.X,
                        op=mybir.AluOpType.add,
                    ).then_inc(vsem[b], 1)

    nc.compile()
    return nc


def kernel(h, src, dst):
    global LAST_RESULT
    h = np.asarray(h, dtype=np.float32)
    hp = np.zeros((NPAD, D), np.float32)
    hp[:N_NODES] = h
    src = np.asarray(src).astype(np.int64)
    dst = np.asarray(dst).astype(np.int64)
    E = src.shape[0]

    schedule, seqs, sidx, didx, u_total, e_total = _host_prep(src, dst)
    in_maps = [
        {"h": hp, "sidx": _wrap_idx(sidx[c]), "didx": _wrap_idx(didx[c])}
        for c in range(N_CORES)
    ]
    nc = _build_nc(schedule, u_total, e_total)

    if TRACE or os.environ.get("BASS_TRACE"):
        _ensure_ntff_hook()
    res = run_bass_kernel_spmd(nc, in_maps, core_ids=list(range(N_CORES)), trace=TRACE)
    LAST_RESULT = res

    out = np.empty(E, np.float32)
    for c in range(N_CORES):
        dots = res.results[c]["out"].T.reshape(-1)
        seq = seqs[c]
        valid = seq >= 0
        out[seq[valid]] = dots[valid]
    return out
